# revision 21
# baseline (speedup 1.0000x reference)
"""Expert-parallel MLP (per-expert FFN + exact GELU + bungee scale + RMS stat)
for 8 Trainium2 NeuronCores.

Reference computation (per expert e):
    h = gelu(x[e] @ w1[e] + b1[e], approximate=False)
    y[e] = (h @ w2[e] + b2[e]) * bungee[e]
outputs: (y, sqrt(mean(y^2)), mean(bungee), min(bungee), max(bungee))

Sharding: experts 4i..4i+3 -> core i.  Host pre-transposes x to x^T[e] =
[D, C] and casts x/w1/w2 to bf16; the device computes everything in the
transposed orientation (h^T, y^T) so both matmul stationary operands are
natural memory slices and biases land on the partition axis.  y^T comes
back per-core and is un-transposed on the host; the scalar RMS is combined
from per-core partial sums of squares at gather time.
"""

import numpy as np

try:
    import concourse  # noqa: F401
except ImportError:  # pragma: no cover - grading env should have it on path
    import sys

    for p in ("/opt/trn_rl_repo", "/root/.axon_site/_ro/trn_rl_repo"):
        if p not in sys.path:
            sys.path.insert(0, p)

from concourse import bacc, bass, mybir, tile
from concourse import bass_utils

# Problem shape (hardcoded per contract)
E, C, D, H = 32, 4096, 256, 1024
N_CORES = 8
E_LOC = E // N_CORES  # 4 experts per core
P = 128
DT = D // P  # 2 d-tiles
HT = H // P  # 8 h-tiles
CCH = 512  # moving free-dim chunk (1 PSUM bank fp32 — matmul N limit)
NCH = C // CCH  # 8 chunks per expert

F32 = mybir.dt.float32
BF16 = mybir.dt.bfloat16
AF = mybir.ActivationFunctionType
ALU = mybir.AluOpType

_CACHED_NC = None
_GELU = AF.Gelu  # overridable for CoreSim (which lacks a Gelu impl)
# debug feature toggles (bisection of HW failures)
_EN_SSQ = True  # per-tile squared-sum accumulation + ssq epilogue
_EN_BSTATS = True  # bungee mean/min/max block


def _emit(nc, tc, ctx):
    xt_d = nc.dram_tensor("xt", (E_LOC, D, C), BF16, kind="ExternalInput").ap()
    w1_d = nc.dram_tensor("w1", (E_LOC, D, H), BF16, kind="ExternalInput").ap()
    w2_d = nc.dram_tensor("w2", (E_LOC, H, D), BF16, kind="ExternalInput").ap()
    b1_d = nc.dram_tensor("b1", (E_LOC, H), F32, kind="ExternalInput").ap()
    b2_d = nc.dram_tensor("b2", (E_LOC, D), F32, kind="ExternalInput").ap()
    sb_d = nc.dram_tensor("bungee_b", (E_LOC, P, 1), F32, kind="ExternalInput").ap()
    bf_d = nc.dram_tensor("bungee_full", (1, E), F32, kind="ExternalInput").ap()

    yt_d = nc.dram_tensor("yt", (E_LOC, D, C), F32, kind="ExternalOutput").ap()
    ssq_d = nc.dram_tensor("ssq", (1, 1), F32, kind="ExternalOutput").ap()
    bst_d = nc.dram_tensor("bstats", (1, 3), F32, kind="ExternalOutput").ap()

    w1_pool = ctx.enter_context(tc.tile_pool(name="w1", bufs=2))
    w2_pool = ctx.enter_context(tc.tile_pool(name="w2", bufs=2))
    bias_pool = ctx.enter_context(tc.tile_pool(name="bias", bufs=2))
    xt_pool = ctx.enter_context(tc.tile_pool(name="xt", bufs=3))
    hg_pool = ctx.enter_context(tc.tile_pool(name="hg", bufs=2))
    y_pool = ctx.enter_context(tc.tile_pool(name="y", bufs=4))
    sq_pool = ctx.enter_context(tc.tile_pool(name="sq", bufs=2))
    st_pool = ctx.enter_context(tc.tile_pool(name="st", bufs=1))
    ph_pool = ctx.enter_context(
        tc.tile_pool(name="ph", bufs=4, space=bass.MemorySpace.PSUM)
    )
    py_pool = ctx.enter_context(
        tc.tile_pool(name="py", bufs=3, space=bass.MemorySpace.PSUM)
    )
    ps_pool = ctx.enter_context(
        tc.tile_pool(name="ps", bufs=1, space=bass.MemorySpace.PSUM)
    )

    # one ssq column per (expert, chunk, d-tile) y tile
    ssq_cols = st_pool.tile([P, E_LOC * NCH * DT], F32, tag="ssqc")

    expert_state = {}
    expert_state2 = {}

    def load_expert_a(e):
        """First-phase loads: what matmul1 needs (w1, b1)."""
        w1_sb = w1_pool.tile([P, DT, H], BF16, tag="w1")
        nc.sync.dma_start(w1_sb[:], w1_d[e].rearrange("(d p) h -> p d h", p=P))
        b1_sb = bias_pool.tile([P, HT], F32, tag="b1")
        nc.sync.dma_start(b1_sb[:], b1_d[e].rearrange("(t p) -> p t", p=P))
        expert_state[e] = (w1_sb, b1_sb)

    def load_expert_b(e):
        """Deferred loads: what matmul2 needs (w2, scaled b2, bungee)."""
        w2_sb = w2_pool.tile([P, HT, D], BF16, tag="w2")
        nc.sync.dma_start(w2_sb[:], w2_d[e].rearrange("(h p) d -> p h d", p=P))
        b2_sb = bias_pool.tile([P, DT], F32, tag="b2")
        nc.sync.dma_start(b2_sb[:], b2_d[e].rearrange("(t p) -> p t", p=P))
        s_sb = bias_pool.tile([P, 1], F32, tag="s")
        nc.sync.dma_start(s_sb[:], sb_d[e])
        b2s_sb = bias_pool.tile([P, DT], F32, tag="b2s")
        nc.vector.tensor_scalar_mul(b2s_sb[:], b2_sb[:], s_sb[:, 0:1])
        expert_state2[e] = (w2_sb, b2s_sb, s_sb)

    def emit_m1(e, ch):
        """matmul1 + gelu for chunk ch of expert e -> returns h^T tile."""
        w1_sb, b1_sb = expert_state[e]
        xt_sb = xt_pool.tile([P, DT, CCH], BF16, tag="xt")
        nc.sync.dma_start(
            xt_sb[:],
            xt_d[e].rearrange("(d p) c -> p d c", p=P)[
                :, :, ch * CCH : (ch + 1) * CCH
            ],
        )
        hg = hg_pool.tile([P, HT, CCH], BF16, tag="hg")
        for h in range(HT):
            ph = ph_pool.tile([P, CCH], F32, tag="ph")
            hb = slice(h * P, (h + 1) * P)
            nc.tensor.matmul(
                ph[:], w1_sb[:, 0, hb], xt_sb[:, 0, :], start=True, stop=False
            )
            nc.tensor.matmul(
                ph[:], w1_sb[:, 1, hb], xt_sb[:, 1, :], start=False, stop=True
            )
            # h^T tile = gelu(h + b1), written as bf16 for matmul2
            nc.scalar.activation(
                hg[:, h, :], ph[:], _GELU, bias=b1_sb[:, h : h + 1]
            )
        return hg

    def emit_m2(e, ch, hg, t_idx):
        """matmul2 + bias/scale drain + ssq for chunk ch of expert e."""
        if e not in expert_state2:
            load_expert_b(e)
        w2_sb, b2s_sb, s_sb = expert_state2[e]
        for d in range(DT):
            py = py_pool.tile([P, CCH], F32, tag="py")
            db = slice(d * P, (d + 1) * P)
            for h in range(HT):
                nc.tensor.matmul(
                    py[:],
                    w2_sb[:, h, db],
                    hg[:, h, :],
                    start=(h == 0),
                    stop=(h == HT - 1),
                )
            y_sb = y_pool.tile([P, CCH], F32, tag="y")
            # y = (py + b2) * s  ==  py * s + b2*s   (DVE, PSUM drain)
            nc.vector.tensor_scalar(
                y_sb[:], py[:], s_sb[:, 0:1], b2s_sb[:, d : d + 1],
                op0=ALU.mult, op1=ALU.add,
            )
            if _EN_SSQ:
                sq = sq_pool.tile([P, CCH], F32, tag="sq")
                nc.vector.tensor_mul(sq[:], y_sb[:], y_sb[:])
                nc.vector.reduce_sum(
                    ssq_cols[:, 2 * t_idx + d : 2 * t_idx + d + 1], sq[:],
                    axis=mybir.AxisListType.X,
                )
            nc.sync.dma_start(
                yt_d[e].rearrange("(d p) c -> p d c", p=P)[
                    :, d, ch * CCH : (ch + 1) * CCH
                ],
                y_sb[:],
            )

    # software-pipelined by one chunk so PE never waits on the gelu drain
    tasks = [(e, ch) for e in range(E_LOC) for ch in range(NCH)]
    prev = None
    for t, (e, ch) in enumerate(tasks):
        if ch == 0:
            load_expert_a(e)
        hg = emit_m1(e, ch)
        if prev is not None:
            emit_m2(prev[0], prev[1], prev[2], t - 1)
        prev = (e, ch, hg)
    emit_m2(prev[0], prev[1], prev[2], len(tasks) - 1)

    # ---- ssq epilogue: [P, ncols] -> [P,1] -> [1,1] via ones-matmul
    ssq_sb = st_pool.tile([1, 1], F32, tag="ssqo")
    if _EN_SSQ:
        ssq_vec = st_pool.tile([P, 1], F32, tag="ssqv")
        nc.vector.reduce_sum(ssq_vec[:], ssq_cols[:], axis=mybir.AxisListType.X)
        ones_sb = st_pool.tile([P, 1], F32, tag="ones")
        nc.vector.memset(ones_sb[:], 1.0)
        ps = ps_pool.tile([1, 1], F32, tag="pss")
        nc.tensor.matmul(ps[:], ones_sb[:], ssq_vec[:], start=True, stop=True)
        nc.scalar.copy(ssq_sb[:], ps[:])
    else:
        nc.vector.memset(ssq_sb[:], 0.0)
    nc.sync.dma_start(ssq_d[:], ssq_sb[:])

    # ---- bungee stats (same on every core): mean / min / max of full bungee
    bst = st_pool.tile([1, 3], F32, tag="bst")
    if _EN_BSTATS:
        bt = st_pool.tile([1, E], F32, tag="bt")
        nc.sync.dma_start(bt[:], bf_d[:])
        bsum = st_pool.tile([1, 1], F32, tag="bsum")
        nc.vector.reduce_sum(bsum[:], bt[:], axis=mybir.AxisListType.X)
        nc.vector.tensor_scalar_mul(bst[:, 0:1], bsum[:], 1.0 / E)
        nc.vector.tensor_reduce(bst[:, 1:2], bt[:], mybir.AxisListType.X, ALU.min)
        nc.vector.tensor_reduce(bst[:, 2:3], bt[:], mybir.AxisListType.X, ALU.max)
    else:
        nc.vector.memset(bst[:], 0.0)
    nc.sync.dma_start(bst_d[:], bst[:])


def _build():
    global _CACHED_NC
    if _CACHED_NC is not None:
        return _CACHED_NC
    from contextlib import ExitStack

    nc = bacc.Bacc("TRN2", target_bir_lowering=False, debug=False)
    with tile.TileContext(nc) as tc:
        with ExitStack() as ctx:
            _emit(nc, tc, ctx)
    nc.compile()
    _CACHED_NC = nc
    return nc


def kernel(x, w1, b1, w2, b2, bungee):
    nc = _build()
    bf16 = mybir.dt.np(BF16)

    xb = np.asarray(x, np.float32).astype(bf16)  # [E, C, D]
    w1b = np.asarray(w1, np.float32).astype(bf16)
    w2b = np.asarray(w2, np.float32).astype(bf16)
    b1f = np.ascontiguousarray(np.asarray(b1, np.float32))
    b2f = np.ascontiguousarray(np.asarray(b2, np.float32))
    bgf = np.asarray(bungee, np.float32)
    bfull = np.ascontiguousarray(bgf.reshape(1, E))

    in_maps = []
    for i in range(N_CORES):
        sl = slice(i * E_LOC, (i + 1) * E_LOC)
        in_maps.append(
            {
                "xt": np.ascontiguousarray(xb[sl].transpose(0, 2, 1)),
                "w1": np.ascontiguousarray(w1b[sl]),
                "w2": np.ascontiguousarray(w2b[sl]),
                "b1": b1f[sl],
                "b2": b2f[sl],
                "bungee_b": np.ascontiguousarray(
                    np.repeat(bgf[sl, None], P, axis=1)[:, :, None]
                ),
                "bungee_full": bfull,
            }
        )

    global _LAST_IN_MAPS
    _LAST_IN_MAPS = in_maps
    res = bass_utils.run_bass_kernel_spmd(
        nc, in_maps, core_ids=list(range(N_CORES))
    )

    y = np.empty((E, C, D), np.float32)
    ssq_total = 0.0
    for i in range(N_CORES):
        out = res.results[i]
        y[i * E_LOC : (i + 1) * E_LOC] = out["yt"].transpose(0, 2, 1)
        ssq_total += float(out["ssq"][0, 0])
    rms = np.float32(np.sqrt(ssq_total / (E * C * D)))
    bst = res.results[0]["bstats"]
    b_mean = np.float32(bst[0, 0])
    b_min = np.float32(bst[0, 1])
    b_max = np.float32(bst[0, 2])
    return (y, rms, b_mean, b_min, b_max)


# revision 29
# speedup vs baseline: 1.4482x; 1.4482x over previous
"""Expert-parallel MLP (per-expert FFN + exact GELU + bungee scale + RMS stat)
for 8 Trainium2 NeuronCores.

Reference computation (per expert e):
    h = gelu(x[e] @ w1[e] + b1[e], approximate=False)
    y[e] = (h @ w2[e] + b2[e]) * bungee[e]
outputs: (y, sqrt(mean(y^2)), mean(bungee), min(bungee), max(bungee))

Sharding: experts 4i..4i+3 -> core i.  Host pre-transposes x to x^T[e] =
[D, C] and casts x/w1/w2 to bf16; the device computes everything in the
transposed orientation (h^T, y^T) so both matmul stationary operands are
natural memory slices and biases land on the partition axis.  y^T comes
back per-core and is un-transposed on the host; the scalar RMS is combined
from per-core partial sums of squares at gather time.
"""

import numpy as np

try:
    import concourse  # noqa: F401
except ImportError:  # pragma: no cover - grading env should have it on path
    import sys

    for p in ("/opt/trn_rl_repo", "/root/.axon_site/_ro/trn_rl_repo"):
        if p not in sys.path:
            sys.path.insert(0, p)

from concourse import bacc, bass, mybir, tile
from concourse import bass_utils

# Problem shape (hardcoded per contract)
E, C, D, H = 32, 4096, 256, 1024
N_CORES = 8
E_LOC = E // N_CORES  # 4 experts per core
P = 128
DT = D // P  # 2 d-tiles
HT = H // P  # 8 h-tiles
CCH = 512  # moving free-dim chunk (1 PSUM bank fp32 — matmul N limit)
NCH = C // CCH  # 8 chunks per expert

F32 = mybir.dt.float32
BF16 = mybir.dt.bfloat16
AF = mybir.ActivationFunctionType
ALU = mybir.AluOpType

_CACHED_NC = None
_GELU = AF.Gelu  # overridable for CoreSim (which lacks a Gelu impl)
_FP8 = False  # fp8e4 DoubleRow matmuls instead of bf16
# debug feature toggles (bisection of HW failures)
_EN_SSQ = True  # per-tile squared-sum accumulation + ssq epilogue
_EN_BSTATS = True  # bungee mean/min/max block


def _emit(nc, tc, ctx):
    XDT = mybir.dt.float8e4 if _FP8 else BF16
    DR = mybir.MatmulPerfMode.DoubleRow
    xt_d = nc.dram_tensor("xt", (E_LOC, D, C), XDT, kind="ExternalInput").ap()
    w1_d = nc.dram_tensor("w1", (E_LOC, D, H), XDT, kind="ExternalInput").ap()
    w2_d = nc.dram_tensor("w2", (E_LOC, H, D), XDT, kind="ExternalInput").ap()
    b1_d = nc.dram_tensor("b1", (E_LOC, H), F32, kind="ExternalInput").ap()
    b2_d = nc.dram_tensor("b2", (E_LOC, D), F32, kind="ExternalInput").ap()
    sb_d = nc.dram_tensor("bungee_b", (E_LOC, P, 1), F32, kind="ExternalInput").ap()
    bf_d = nc.dram_tensor("bungee_full", (1, E), F32, kind="ExternalInput").ap()

    yt_d = nc.dram_tensor("yt", (E_LOC, D, C), F32, kind="ExternalOutput").ap()
    ssq_d = nc.dram_tensor("ssq", (1, 1), F32, kind="ExternalOutput").ap()
    bst_d = nc.dram_tensor("bstats", (1, 3), F32, kind="ExternalOutput").ap()

    w1_pool = ctx.enter_context(tc.tile_pool(name="w1", bufs=2))
    w2_pool = ctx.enter_context(tc.tile_pool(name="w2", bufs=2))
    bias_pool = ctx.enter_context(tc.tile_pool(name="bias", bufs=2))
    xt_pool = ctx.enter_context(tc.tile_pool(name="xt", bufs=3))
    hg_pool = ctx.enter_context(tc.tile_pool(name="hg", bufs=2))
    y_pool = ctx.enter_context(tc.tile_pool(name="y", bufs=4))
    sq_pool = ctx.enter_context(tc.tile_pool(name="sq", bufs=2))
    st_pool = ctx.enter_context(tc.tile_pool(name="st", bufs=1))
    ph_pool = ctx.enter_context(
        tc.tile_pool(name="ph", bufs=4, space=bass.MemorySpace.PSUM)
    )
    py_pool = ctx.enter_context(
        tc.tile_pool(name="py", bufs=3, space=bass.MemorySpace.PSUM)
    )
    ps_pool = ctx.enter_context(
        tc.tile_pool(name="ps", bufs=1, space=bass.MemorySpace.PSUM)
    )

    # one ssq column per (expert, chunk, d-tile) y tile
    ssq_cols = st_pool.tile([P, E_LOC * NCH * DT], F32, tag="ssqc")

    expert_state = {}
    expert_state2 = {}

    def load_expert_a(e):
        """First-phase loads: what matmul1 needs (w1, b1)."""
        w1_sb = w1_pool.tile([P, DT, H], XDT, tag="w1")
        nc.sync.dma_start(w1_sb[:], w1_d[e].rearrange("(d p) h -> p d h", p=P))
        b1_sb = bias_pool.tile([P, HT], F32, tag="b1")
        nc.sync.dma_start(b1_sb[:], b1_d[e].rearrange("(t p) -> p t", p=P))
        expert_state[e] = (w1_sb, b1_sb)

    def load_expert_b(e):
        """Deferred loads: what matmul2 needs (w2, scaled b2, bungee)."""
        w2_sb = w2_pool.tile([P, HT, D], XDT, tag="w2")
        nc.sync.dma_start(w2_sb[:], w2_d[e].rearrange("(h p) d -> p h d", p=P))
        b2_sb = bias_pool.tile([P, DT], F32, tag="b2")
        nc.sync.dma_start(b2_sb[:], b2_d[e].rearrange("(t p) -> p t", p=P))
        s_sb = bias_pool.tile([P, 1], F32, tag="s")
        nc.sync.dma_start(s_sb[:], sb_d[e])
        b2s_sb = bias_pool.tile([P, DT], F32, tag="b2s")
        nc.vector.tensor_scalar_mul(b2s_sb[:], b2_sb[:], s_sb[:, 0:1])
        expert_state2[e] = (w2_sb, b2s_sb, s_sb)

    def emit_m1(e, ch):
        """matmul1 + gelu for chunk ch of expert e -> returns h^T tile."""
        w1_sb, b1_sb = expert_state[e]
        xt_sb = xt_pool.tile([P, DT, CCH], XDT, tag="xt")
        nc.sync.dma_start(
            xt_sb[:],
            xt_d[e].rearrange("(d p) c -> p d c", p=P)[
                :, :, ch * CCH : (ch + 1) * CCH
            ],
        )
        hg = hg_pool.tile([P, HT, CCH], XDT, tag="hg")
        for h in range(HT):
            ph = ph_pool.tile([P, CCH], F32, tag="ph")
            hb = slice(h * P, (h + 1) * P)
            if _FP8:
                # DoubleRow: contraction (k=128)x(i=2) == D=256 in one MM
                nc.tensor.matmul(
                    ph[:], w1_sb[:, :, hb], xt_sb[:, :, :],
                    start=True, stop=True, perf_mode=DR,
                )
            else:
                nc.tensor.matmul(
                    ph[:], w1_sb[:, 0, hb], xt_sb[:, 0, :], start=True, stop=False
                )
                nc.tensor.matmul(
                    ph[:], w1_sb[:, 1, hb], xt_sb[:, 1, :], start=False, stop=True
                )
            # h^T tile = gelu(h + b1), written as bf16 for matmul2
            nc.scalar.activation(
                hg[:, h, :], ph[:], _GELU, bias=b1_sb[:, h : h + 1]
            )
        return hg

    def emit_m2(e, ch, hg, t_idx):
        """matmul2 + bias/scale drain + ssq for chunk ch of expert e."""
        if e not in expert_state2:
            load_expert_b(e)
        w2_sb, b2s_sb, s_sb = expert_state2[e]
        for d in range(DT):
            py = py_pool.tile([P, CCH], F32, tag="py")
            db = slice(d * P, (d + 1) * P)
            if _FP8:
                for j in range(HT // 2):
                    nc.tensor.matmul(
                        py[:],
                        w2_sb[:, 2 * j : 2 * j + 2, db],
                        hg[:, 2 * j : 2 * j + 2, :],
                        start=(j == 0),
                        stop=(j == HT // 2 - 1),
                        perf_mode=DR,
                    )
            else:
                for h in range(HT):
                    nc.tensor.matmul(
                        py[:],
                        w2_sb[:, h, db],
                        hg[:, h, :],
                        start=(h == 0),
                        stop=(h == HT - 1),
                    )
            y_sb = y_pool.tile([P, CCH], F32, tag="y")
            # y = (py + b2) * s  ==  py * s + b2*s   (DVE, PSUM drain)
            nc.vector.tensor_scalar(
                y_sb[:], py[:], s_sb[:, 0:1], b2s_sb[:, d : d + 1],
                op0=ALU.mult, op1=ALU.add,
            )
            if _EN_SSQ:
                sq = sq_pool.tile([P, CCH], F32, tag="sq")
                nc.vector.tensor_mul(sq[:], y_sb[:], y_sb[:])
                nc.vector.reduce_sum(
                    ssq_cols[:, 2 * t_idx + d : 2 * t_idx + d + 1], sq[:],
                    axis=mybir.AxisListType.X,
                )
            nc.sync.dma_start(
                yt_d[e].rearrange("(d p) c -> p d c", p=P)[
                    :, d, ch * CCH : (ch + 1) * CCH
                ],
                y_sb[:],
            )

    # software-pipelined by one chunk so PE never waits on the gelu drain
    tasks = [(e, ch) for e in range(E_LOC) for ch in range(NCH)]
    prev = None
    for t, (e, ch) in enumerate(tasks):
        if ch == 0:
            load_expert_a(e)
        hg = emit_m1(e, ch)
        if prev is not None:
            emit_m2(prev[0], prev[1], prev[2], t - 1)
        prev = (e, ch, hg)
    emit_m2(prev[0], prev[1], prev[2], len(tasks) - 1)

    # ---- ssq epilogue: [P, ncols] -> [P,1] -> [1,1] via ones-matmul
    ssq_sb = st_pool.tile([1, 1], F32, tag="ssqo")
    if _EN_SSQ:
        ssq_vec = st_pool.tile([P, 1], F32, tag="ssqv")
        nc.vector.reduce_sum(ssq_vec[:], ssq_cols[:], axis=mybir.AxisListType.X)
        ones_sb = st_pool.tile([P, 1], F32, tag="ones")
        nc.vector.memset(ones_sb[:], 1.0)
        ps = ps_pool.tile([1, 1], F32, tag="pss")
        nc.tensor.matmul(ps[:], ones_sb[:], ssq_vec[:], start=True, stop=True)
        nc.scalar.copy(ssq_sb[:], ps[:])
    else:
        nc.vector.memset(ssq_sb[:], 0.0)
    nc.sync.dma_start(ssq_d[:], ssq_sb[:])

    # ---- bungee stats (same on every core): mean / min / max of full bungee
    bst = st_pool.tile([1, 3], F32, tag="bst")
    if _EN_BSTATS:
        bt = st_pool.tile([1, E], F32, tag="bt")
        nc.sync.dma_start(bt[:], bf_d[:])
        bsum = st_pool.tile([1, 1], F32, tag="bsum")
        nc.vector.reduce_sum(bsum[:], bt[:], axis=mybir.AxisListType.X)
        nc.vector.tensor_scalar_mul(bst[:, 0:1], bsum[:], 1.0 / E)
        nc.vector.tensor_reduce(bst[:, 1:2], bt[:], mybir.AxisListType.X, ALU.min)
        nc.vector.tensor_reduce(bst[:, 2:3], bt[:], mybir.AxisListType.X, ALU.max)
    else:
        nc.vector.memset(bst[:], 0.0)
    nc.sync.dma_start(bst_d[:], bst[:])


def _build():
    global _CACHED_NC
    if _CACHED_NC is not None:
        return _CACHED_NC
    from contextlib import ExitStack

    nc = bacc.Bacc("TRN2", target_bir_lowering=False, debug=False)
    with tile.TileContext(nc) as tc:
        with ExitStack() as ctx:
            _emit(nc, tc, ctx)
    nc.compile()
    _CACHED_NC = nc
    return nc


def kernel(x, w1, b1, w2, b2, bungee):
    nc = _build()
    xdt = mybir.dt.np(mybir.dt.float8e4 if _FP8 else BF16)

    xb = np.asarray(x, np.float32).astype(xdt)  # [E, C, D]
    w1b = np.asarray(w1, np.float32).astype(xdt)
    w2b = np.asarray(w2, np.float32).astype(xdt)
    b1f = np.ascontiguousarray(np.asarray(b1, np.float32))
    b2f = np.ascontiguousarray(np.asarray(b2, np.float32))
    bgf = np.asarray(bungee, np.float32)
    bfull = np.ascontiguousarray(bgf.reshape(1, E))

    in_maps = []
    for i in range(N_CORES):
        sl = slice(i * E_LOC, (i + 1) * E_LOC)
        in_maps.append(
            {
                "xt": np.ascontiguousarray(xb[sl].transpose(0, 2, 1)),
                "w1": np.ascontiguousarray(w1b[sl]),
                "w2": np.ascontiguousarray(w2b[sl]),
                "b1": b1f[sl],
                "b2": b2f[sl],
                "bungee_b": np.ascontiguousarray(
                    np.repeat(bgf[sl, None], P, axis=1)[:, :, None]
                ),
                "bungee_full": bfull,
            }
        )

    global _LAST_IN_MAPS
    _LAST_IN_MAPS = in_maps
    res = bass_utils.run_bass_kernel_spmd(
        nc, in_maps, core_ids=list(range(N_CORES))
    )

    y = np.empty((E, C, D), np.float32)
    ssq_total = 0.0
    for i in range(N_CORES):
        out = res.results[i]
        y[i * E_LOC : (i + 1) * E_LOC] = out["yt"].transpose(0, 2, 1)
        ssq_total += float(out["ssq"][0, 0])
    rms = np.float32(np.sqrt(ssq_total / (E * C * D)))
    bst = res.results[0]["bstats"]
    b_mean = np.float32(bst[0, 0])
    b_min = np.float32(bst[0, 1])
    b_max = np.float32(bst[0, 2])
    return (y, rms, b_mean, b_min, b_max)
